# revision 4
# baseline (speedup 1.0000x reference)
"""Masked multi-head attention block (B=4, N=1024, D=1024, H=16, DH=64) on 8
Trainium2 NeuronCores.

Sharding: core (b, g) = 2*b + g handles batch b and head-group g (8 of 16
heads). Each core computes qkv projections for its heads, attention, and its
partial output projection; the host sums the two head-group partials per batch.

Mask handling: the host gathers only the valid tokens per batch (padded to a
multiple of 128 with key-bias -30000 on the pad), so the device computes a
dense unmasked attention over ~half the sequence; invalid token rows of the
output are b_out.

v2 layout (per core, Vp = padded key count, Vq = padded query count):
  All inputs bf16.  xT [D,Vp], wqk [D,1024] (q pre-scaled), wv [D,512],
  wo [512,D].
  V'   [Vp, 8, 65]  values per head + ones column (softmax denominator)
  qkT  via lhsT=wqk chunks, rhs=xT; evacuated to bf16 SBUF
  S^T  [Vp,Vq]/head = K_chunk @ Q^T; exp on ACT with per-partition pad bias;
       the two heads of a pair are emitted adjacently on disjoint PE row
       groups (rows 0-63 / 64-127) so their matmuls overlap.
  O^T  [65, Vq]/head = V'^T @ P^T accumulated over key chunks (row 64=denom)
  norm: 1/denom on DVE (reciprocal), partition-broadcast via a K=1
        ones-matmul on the PE into PSUM, multiply on DVE -> A^T bf16.
  y    [Vq, D] = sum_h A_h @ w_out_h  (K=128 accumulating matmuls)

The qk projection pairs 1-3 are interleaved as background PE work inside the
attention pipeline so the PE stays busy (and at warm clock) while ACT runs
the exp instructions.  PSUM during stage 2: qkps 2 banks + stps 4 + otps 2.
"""
import json
import os
import sys

import numpy as np

sys.path.insert(0, "/opt/trn_rl_repo")

import concourse.bass as bass
import concourse.mybir as mybir
from concourse.tile import TileContext
from concourse import bass_utils

F32 = mybir.dt.float32
F32R = mybir.dt.float32r
BF16 = mybir.dt.bfloat16
AF = mybir.ActivationFunctionType

B, N, D, H, DH = 4, 1024, 1024, 16, 64
NCORES = 8
PAD_BIAS = -30000.0


def _install_patches():
    """The walrus build in this container accepts only one semaphore wait per
    instruction; hoist extra waits onto same-engine NoOps in the BIR json."""
    if getattr(bass.Bass, "_split_waits_patched", False):
        return
    orig = bass.Bass.to_json_bytes

    def to_json_bytes_split(self, *a, **k):
        j = json.loads(orig(self, *a, **k))
        for fn in j.get("functions", []):
            for bb in fn.get("blocks", []):
                out = []
                for ins in bb.get("instructions", []):
                    si = ins.get("sync_info") or {}
                    waits = si.get("on_wait") or []
                    if len(waits) > 1:
                        for i, w in enumerate(waits[:-1]):
                            out.append({
                                "debug": ins.get("debug", 0),
                                "engine": ins["engine"],
                                "ins": [],
                                "name": f"{ins['name']}_sw{i}",
                                "opcode": "NoOp",
                                "outs": [],
                                "text_hint": "splitw",
                                "sync_info": {"on_update": [], "on_wait": [w]},
                            })
                        si["on_wait"] = [waits[-1]]
                    out.append(ins)
                bb["instructions"] = out
        return json.dumps(j).encode()

    bass.Bass.to_json_bytes = to_json_bytes_split

    def _drain_and_barrier(self, tick_clock, wait_clock):
        import re as _re
        import bass_rust as _br
        from concourse.vector_clock import ScopedClock as _SC
        gc = tick_clock.global_clock
        comps = eval(_re.match(r"VectorClock\((\[.*\])\)", repr(gc)).group(1))
        for i, v in enumerate(comps):
            if v <= 0:
                continue
            sub = [0] * len(comps)
            sub[i] = v
            nop = self.nc.sync.nop(nofuse=True, hint="final_wait")
            wait_clock.add_sem_waits(nop.ins, _SC({None: _br.VectorClock(sub)}))
        self.nc.sync.drain()
        self.nc.all_engine_barrier()
        assert self.sems is not None
        popped = self.nc._tile_sem_poison_stack.pop()
        assert popped is self._sem_poison
        self.nc.clear_and_free_semaphores(list(self.sems.allocated().values()))

    TileContext._drain_and_barrier = _drain_and_barrier
    bass.Bass._split_waits_patched = True


def _build_program(Vp, Vq):
    KC = Vp // 128
    # key-side and query-side column slices: each slice gets its own PSUM
    # bank (matmul output must not cross a bank) and stays >= 256 wide.
    W = Vp if Vp <= 512 else Vp // 2
    QS = [(i * W, W) for i in range(Vp // W)]
    NQ = len(QS)
    Wq = Vq if Vq <= 512 else Vq // 2
    QSq = [(i * Wq, Wq) for i in range(Vq // Wq)]
    NQq = len(QSq)
    KCq = -(-Vq // 128)  # query-row chunks for the output projection
    NS = [(0, 512), (512, 512)]  # output D column halves

    nc = bass.Bass(trn_type="TRN2", target_bir_lowering=False, debug=False,
                   num_devices=NCORES)
    xt = nc.declare_dram_parameter("xt", [D, Vp], BF16, isOutput=False).ap()
    wqk = nc.declare_dram_parameter("wqk", [D, 1024], BF16, isOutput=False).ap()
    wv = nc.declare_dram_parameter("wv", [D, 512], BF16, isOutput=False).ap()
    wo = nc.declare_dram_parameter("wo", [512, D], BF16, isOutput=False).ap()
    biasv = nc.declare_dram_parameter("biasv", [128, KC], F32, isOutput=False).ap()
    onesr = nc.declare_dram_parameter("onesr", [1, 64], F32R, isOutput=False).ap()
    y = nc.declare_dram_parameter("y", [Vq, D], F32, isOutput=True).ap()

    with TileContext(nc) as tc:
        with tc.tile_pool(name="consts", bufs=1) as consts, \
             tc.tile_pool(name="xsb", bufs=1) as xpool, \
             tc.tile_pool(name="wqk", bufs=1) as wqkpool, \
             tc.tile_pool(name="wv", bufs=1) as wvpool, \
             tc.tile_pool(name="wo", bufs=1) as wopool, \
             tc.tile_pool(name="qk", bufs=1) as qkpool, \
             tc.tile_pool(name="vp", bufs=1) as vppool, \
             tc.tile_pool(name="pt", bufs=2 * KC + 6) as ptpool, \
             tc.tile_pool(name="at", bufs=1) as atpool, \
             tc.tile_pool(name="norm", bufs=3) as npool, \
             tc.tile_pool(name="ysb", bufs=2) as ypool:

            bias_sb = consts.tile([128, KC], F32)
            ones_sb = consts.tile([1, 64], F32R)
            nc.sync.dma_start(out=bias_sb[:], in_=biasv[:])
            nc.sync.dma_start(out=ones_sb[:], in_=onesr[:])

            # ---- input DMAs: x + wv first (V' needs them), then wqk, wo ----
            xsb, wqk_sb, wv_sb, wo_sb = [], [], [], []
            for k in range(8):
                t = xpool.tile([128, Vp], BF16, tag=f"x{k}", name=f"x_{k}")
                nc.scalar.dma_start(out=t[:], in_=xt[k * 128:(k + 1) * 128, :])
                xsb.append(t)
            for k in range(8):
                wt = wvpool.tile([128, 512], BF16, tag=f"wv{k}", name=f"wv_{k}")
                nc.scalar.dma_start(out=wt[:], in_=wv[k * 128:(k + 1) * 128, :])
                wv_sb.append(wt)
            for k in range(8):
                wt = wqkpool.tile([128, 1024], BF16, tag=f"wqk{k}",
                                  name=f"wqk_{k}")
                nc.sync.dma_start(out=wt[:], in_=wqk[k * 128:(k + 1) * 128, :])
                wqk_sb.append(wt)
            for j in range(4):
                wt = wopool.tile([128, 1024], BF16, tag=f"wo{j}")
                nc.sync.dma_start(out=wt[:], in_=wo[j * 128:(j + 1) * 128, :])
                wo_sb.append(wt)

            at2 = [atpool.tile([128, Vq], BF16, tag=f"at{j}", name=f"at2_{j}")
                   for j in range(4)]
            qk_sb = [None] * 8
            vp_sb = []

            # ---- V' tiles [128, 8, 65] per key chunk (own PSUM scope) ----
            with tc.tile_pool(name="vps", bufs=2, space="PSUM") as vps:
                for c in range(KC):
                    ps = vps.tile([128, 512], F32, tag="vpp")
                    for k in range(8):
                        nc.tensor.matmul(
                            ps[:],
                            lhsT=xsb[k][:, c * 128:(c + 1) * 128],
                            rhs=wv_sb[k][:],
                            start=(k == 0), stop=(k == 7))
                    vt = vppool.tile([128, 8, 65], BF16, tag=f"vp{c}")
                    nc.vector.tensor_copy(
                        out=vt[:, :, 0:64],
                        in_=ps[:].rearrange("p (h d) -> p h d", h=8))
                    nc.gpsimd.memset(vt[:, :, 64:65], 1.0)
                    vp_sb.append(vt)

            with tc.tile_pool(name="qkps", bufs=1, space="PSUM") as qkps, \
                 tc.tile_pool(name="stps", bufs=2, space="PSUM") as stps, \
                 tc.tile_pool(name="otps", bufs=1, space="PSUM") as otps:

                def make_pair_items(p):
                    """Small PE work items for qk projection pair p (m=p is
                    the q chunk, m=4+p the k chunk). PSUM tile allocation is
                    lazy so ring order matches use order."""
                    items = []
                    for m in (p, 4 + p):
                        isq = m < 4
                        mQS, mNQ, mV = (QSq, NQq, Vq) if isq else (QS, NQ, Vp)
                        mW = Wq if isq else W
                        hold = {}
                        for qi, (n0, nw) in enumerate(mQS):
                            for k0 in (0, 4):
                                def mm4(m=m, qi=qi, n0=n0, nw=nw, k0=k0,
                                        hold=hold):
                                    if "ps" not in hold:
                                        hold["ps"] = qkps.tile(
                                            [128, 2, 512], F32, tag="qkp",
                                            name=f"qkp_{m}")
                                    for k in range(k0, k0 + 4):
                                        nc.tensor.matmul(
                                            hold["ps"][:, qi, 0:nw],
                                            lhsT=wqk_sb[k][:,
                                                           m * 128:(m + 1) * 128],
                                            rhs=xsb[k][:, n0:n0 + nw],
                                            start=(k == 0), stop=(k == 7))
                                items.append(mm4)

                        def evac(m=m, hold=hold, mV=mV, mNQ=mNQ, mW=mW):
                            qt = qkpool.tile([128, mV], BF16, tag=f"qk{m}",
                                             name=f"qk_{m}")
                            qtv = qt[:].rearrange("p (q w) -> p q w", q=mNQ)
                            nc.vector.tensor_copy(out=qtv,
                                                  in_=hold["ps"][:, :, 0:mW])
                            qk_sb[m] = qt
                        items.append(evac)
                    return items

                # prologue: qk pair 0 emitted inline
                for item in make_pair_items(0):
                    item()

                # background PE work: qk pairs 1-3, drip-fed into stage 2
                bg = []
                for p in (1, 2, 3):
                    bg.extend(make_pair_items(p))
                bg_i = [0]

                def inject(n):
                    for _ in range(n):
                        if bg_i[0] < len(bg):
                            bg[bg_i[0]]()
                            bg_i[0] += 1

                # ---- stage 2: per head-pair attention pipeline ----
                for hp in range(4):
                    qt = qk_sb[hp]
                    kt = qk_sb[4 + hp]
                    pts = {0: [None] * KC, 1: [None] * KC}
                    ot = {}

                    def emit_ot(h, sub, c, ot=ot, pts=pts):
                        for qi, (n0, nw) in enumerate(QSq):
                            nc.tensor.matmul(
                                ot[sub][:, qi, 0:nw],
                                lhsT=vp_sb[c][:, h, :],
                                rhs=pts[sub][c][:, n0:n0 + nw],
                                start=(c == 0), stop=(c == KC - 1))

                    def emit_norm(h, sub, hp=hp, ot=ot):
                        # numerators -> SBUF; 1/denom on DVE; broadcast the
                        # reciprocal row across 64 partitions with a K=1
                        # ones-matmul; multiply -> A^T (bf16).
                        osb = npool.tile([64, Vq], F32, tag="osb")
                        nc.vector.tensor_copy(
                            out=osb[:].rearrange("p (q w) -> p q w", q=NQq),
                            in_=ot[sub][0:64, :, 0:Wq])
                        rc = npool.tile([1, Vq], F32R, tag="rc")
                        with nc.allow_low_precision(
                                reason="f32r output is bit-identical f32"):
                            nc.vector.reciprocal(
                                out=rc[:].rearrange("p (q w) -> p q w",
                                                    q=NQq),
                                in_=ot[sub][64:65, :, 0:Wq])
                        rb = otps.tile([65, NQq, 512], F32, tag="ot",
                                       name=f"rb_{h}")
                        for qi, (n0, nw) in enumerate(QSq):
                            nc.tensor.matmul(rb[0:64, qi, 0:nw],
                                             lhsT=ones_sb[:],
                                             rhs=rc[0:1, n0:n0 + nw],
                                             start=True, stop=True)
                        rbv = rb[0:64, :, 0:Wq]
                        ov = osb[:].rearrange("p (q w) -> p q w", q=NQq)
                        if sub == 0:
                            nc.vector.tensor_mul(
                                at2[hp][0:64, :].rearrange(
                                    "p (q w) -> p q w", q=NQq), ov, rbv)
                        else:
                            tmp = npool.tile([64, Vq], BF16, tag="odd")
                            nc.vector.tensor_mul(
                                tmp[:].rearrange("p (q w) -> p q w", q=NQq),
                                ov, rbv)
                            nc.sync.dma_start(out=at2[hp][64:128, :],
                                              in_=tmp[:])

                    # S^T + EXP pipeline over key chunks; even head's O^T
                    # trails by 2 chunks; odd head's O^T runs after.
                    for c in range(KC):
                        st = {}
                        for sub in (0, 1):
                            st[sub] = stps.tile([128, NQq, 512], F32,
                                                tag="st",
                                                name=f"st_{hp}_{sub}_{c}")
                        # adjacent emission on disjoint PE row groups
                        for qi, (n0, nw) in enumerate(QSq):
                            for sub in (0, 1):
                                lo = sub * 64
                                nc.tensor.matmul(
                                    st[sub][:, qi, 0:nw],
                                    lhsT=kt[lo:lo + 64,
                                            c * 128:(c + 1) * 128],
                                    rhs=qt[lo:lo + 64, n0:n0 + nw],
                                    start=True, stop=True)
                        for sub in (0, 1):
                            pt = ptpool.tile([128, Vq], BF16, tag="pt",
                                             name=f"pt_{hp}_{sub}_{c}")
                            nc.scalar.activation(
                                out=pt[:].rearrange("p (q w) -> p q w",
                                                    q=NQq),
                                in_=st[sub][:, :, 0:Wq], func=AF.Exp,
                                bias=bias_sb[:, c:c + 1], scale=1.0)
                            pts[sub][c] = pt
                        if c == 0:
                            ot[0] = otps.tile([65, NQq, 512], F32, tag="ot",
                                              name=f"ot_{2 * hp}")
                        if c >= 2:
                            emit_ot(2 * hp, 0, c - 2)
                        inject(2)

                    for cc in range(max(0, KC - 2), KC):
                        emit_ot(2 * hp, 0, cc)
                    emit_norm(2 * hp, 0)
                    inject(1)
                    ot[1] = otps.tile([65, NQq, 512], F32, tag="ot",
                                      name=f"ot_{2 * hp + 1}")
                    for c in range(KC):
                        emit_ot(2 * hp + 1, 1, c)
                        if c % 2 == 1:
                            inject(1)
                    emit_norm(2 * hp + 1, 1)
                    inject(1)
                inject(len(bg))

            # ---- stage 3: y[qc] = sum_j Apair_j @ wopair_j  (K=128) ----
            with tc.tile_pool(name="yps", bufs=2, space="PSUM") as yps:
                for qc in range(KCq):
                    mw = min(128, Vq - qc * 128)
                    yp = yps.tile([128, 1024], F32, tag="yp")
                    for j in range(4):
                        for (n0, nw) in NS:
                            nc.tensor.matmul(
                                yp[0:mw, n0:n0 + nw],
                                lhsT=at2[j][:, qc * 128:qc * 128 + mw],
                                rhs=wo_sb[j][:, n0:n0 + nw],
                                start=(j == 0), stop=(j == 3))
                    ysb = ypool.tile([128, 1024], F32, tag="ysb")
                    nc.vector.tensor_copy(out=ysb[0:mw, :], in_=yp[0:mw, :])
                    nc.scalar.dma_start(out=y[qc * 128:qc * 128 + mw, :],
                                        in_=ysb[0:mw, :])
    return nc


def kernel(x, mask, w_qkv, w_out, b_out):
    _install_patches()
    from concourse.bass_utils import run_bass_kernel_spmd

    x = np.asarray(x, dtype=np.float32)
    mask = np.asarray(mask, dtype=np.float32)
    w_qkv = np.asarray(w_qkv, dtype=np.float32)
    w_out = np.asarray(w_out, dtype=np.float32)
    b_out = np.asarray(b_out, dtype=np.float32)

    idx = [np.nonzero(mask[b] != 0.0)[0] for b in range(B)]
    nv = [len(i) for i in idx]
    Vp = max(128, int(-(-max(nv) // 128)) * 128)
    Vq = max(128, int(-(-max(nv) // 32)) * 32)
    if max(nv) == 0:
        return np.broadcast_to(b_out, (B, N, D)).astype(np.float32).copy()

    import ml_dtypes
    bf16 = ml_dtypes.bfloat16

    scale = float(DH) ** -0.5
    G = 512  # features per head-group
    wqk_g, wv_g, wo_g = [], [], []
    for g in range(2):
        wq = w_qkv[:, g * G:(g + 1) * G] * scale
        wk = w_qkv[:, 1024 + g * G:1024 + (g + 1) * G]
        wqk_g.append(np.ascontiguousarray(
            np.concatenate([wq, wk], axis=1).astype(bf16)))
        wv_g.append(np.ascontiguousarray(
            w_qkv[:, 2048 + g * G:2048 + (g + 1) * G].astype(bf16)))
        wo_g.append(np.ascontiguousarray(
            w_out[g * G:(g + 1) * G, :].astype(bf16)))

    xt_b, bias_b = [], []
    for b in range(B):
        pad = Vp - nv[b]
        idxp = np.concatenate([idx[b], np.zeros(pad, dtype=np.int64)])
        xg = x[b][idxp, :]
        xt_b.append(np.ascontiguousarray(xg.T.astype(bf16)))
        bv = np.concatenate([
            np.zeros(nv[b], dtype=np.float32),
            np.full(pad, PAD_BIAS, dtype=np.float32)])
        bias_b.append(np.ascontiguousarray(bv.reshape(-1, 128).T))
    onesr = np.ones((1, 64), dtype=np.float32)

    nc = _build_program(Vp, Vq)
    in_maps = []
    for core in range(NCORES):
        b, g = core // 2, core % 2
        in_maps.append({
            "xt": xt_b[b], "wqk": wqk_g[g], "wv": wv_g[g], "wo": wo_g[g],
            "biasv": bias_b[b], "onesr": onesr,
        })

    trace = bool(os.environ.get("BASSK_TRACE"))
    if trace:
        _install_profile_hook()
    res = run_bass_kernel_spmd(nc, in_maps, list(range(NCORES)), trace=trace)
    global last_exec_time_ns
    last_exec_time_ns = res.exec_time_ns

    out = np.zeros((B, N, D), dtype=np.float32)
    for b in range(B):
        yb = res.results[2 * b]["y"] + res.results[2 * b + 1]["y"]
        out[b][idx[b]] = yb[:nv[b]]
    out += b_out
    return out


last_exec_time_ns = None


def _install_profile_hook():
    import types
    import antenv
    if 'antenv.axon_hooks' in sys.modules:
        return
    import trn_agent_boot.trn_boot as tb
    _hook = tb._ntff_profile_via_ctypes('/opt/axon/libaxon_pjrt.so')
    mod = types.ModuleType('antenv.axon_hooks')
    mod.get_axon_ntff_profile_hook = lambda: _hook
    mod.set_axon_ntff_profile_hook = lambda h: None
    sys.modules['antenv.axon_hooks'] = mod
    antenv.axon_hooks = mod
    bass_utils.upload_artifacts = lambda tmpdir: "local://skipped"


# revision 9
# speedup vs baseline: 1.0431x; 1.0431x over previous
"""Masked multi-head attention block (B=4, N=1024, D=1024, H=16, DH=64) on 8
Trainium2 NeuronCores.

Sharding: core (b, g) = 2*b + g handles batch b and head-group g (8 of 16
heads). Each core computes qkv projections for its heads, attention, and its
partial output projection; the host sums the two head-group partials per batch.

Mask handling: the host gathers only the valid tokens per batch (padded to a
multiple of 128 with key-bias -30000 on the pad), so the device computes a
dense unmasked attention over ~half the sequence; invalid token rows of the
output are b_out.

v2 layout (per core, Vp = padded key count, Vq = padded query count):
  All inputs bf16.  xT [D,Vp], wqk [D,1024] (q pre-scaled), wv [D,512],
  wo [512,D].
  V'   [Vp, 8, 65]  values per head + ones column (softmax denominator)
  qkT  via lhsT=wqk chunks, rhs=xT; evacuated to bf16 SBUF
  S^T  [Vp,Vq]/head = K_chunk @ Q^T; exp on ACT with per-partition pad bias;
       the two heads of a pair are emitted adjacently on disjoint PE row
       groups (rows 0-63 / 64-127) so their matmuls overlap.
  O^T  [65, Vq]/head = V'^T @ P^T accumulated over key chunks (row 64=denom)
  norm: 1/denom on DVE (reciprocal), partition-broadcast via a K=1
        ones-matmul on the PE into PSUM, multiply on DVE -> A^T bf16.
  y    [Vq, D] = sum_h A_h @ w_out_h  (K=128 accumulating matmuls)

The qk projection pairs 1-3 are interleaved as background PE work inside the
attention pipeline so the PE stays busy (and at warm clock) while ACT runs
the exp instructions.  PSUM during stage 2: qkps 2 banks + stps 4 + otps 2.
"""
import json
import os
import sys

import numpy as np

sys.path.insert(0, "/opt/trn_rl_repo")

import concourse.bass as bass
import concourse.mybir as mybir
from concourse.tile import TileContext
from concourse import bass_utils

F32 = mybir.dt.float32
F32R = mybir.dt.float32r
BF16 = mybir.dt.bfloat16
AF = mybir.ActivationFunctionType

B, N, D, H, DH = 4, 1024, 1024, 16, 64
NCORES = 8
PAD_BIAS = -30000.0


def _install_patches():
    """The walrus build in this container accepts only one semaphore wait per
    instruction; hoist extra waits onto same-engine NoOps in the BIR json."""
    if getattr(bass.Bass, "_split_waits_patched", False):
        return
    orig = bass.Bass.to_json_bytes

    def to_json_bytes_split(self, *a, **k):
        j = json.loads(orig(self, *a, **k))
        for fn in j.get("functions", []):
            for bb in fn.get("blocks", []):
                out = []
                for ins in bb.get("instructions", []):
                    si = ins.get("sync_info") or {}
                    waits = si.get("on_wait") or []
                    if len(waits) > 1:
                        for i, w in enumerate(waits[:-1]):
                            out.append({
                                "debug": ins.get("debug", 0),
                                "engine": ins["engine"],
                                "ins": [],
                                "name": f"{ins['name']}_sw{i}",
                                "opcode": "NoOp",
                                "outs": [],
                                "text_hint": "splitw",
                                "sync_info": {"on_update": [], "on_wait": [w]},
                            })
                        si["on_wait"] = [waits[-1]]
                    out.append(ins)
                bb["instructions"] = out
        return json.dumps(j).encode()

    bass.Bass.to_json_bytes = to_json_bytes_split

    def _drain_and_barrier(self, tick_clock, wait_clock):
        import re as _re
        import bass_rust as _br
        from concourse.vector_clock import ScopedClock as _SC
        gc = tick_clock.global_clock
        comps = eval(_re.match(r"VectorClock\((\[.*\])\)", repr(gc)).group(1))
        for i, v in enumerate(comps):
            if v <= 0:
                continue
            sub = [0] * len(comps)
            sub[i] = v
            nop = self.nc.sync.nop(nofuse=True, hint="final_wait")
            wait_clock.add_sem_waits(nop.ins, _SC({None: _br.VectorClock(sub)}))
        self.nc.sync.drain()
        self.nc.all_engine_barrier()
        assert self.sems is not None
        popped = self.nc._tile_sem_poison_stack.pop()
        assert popped is self._sem_poison
        self.nc.clear_and_free_semaphores(list(self.sems.allocated().values()))

    TileContext._drain_and_barrier = _drain_and_barrier
    bass.Bass._split_waits_patched = True


def _build_program(Vp, Vq):
    KC = Vp // 128
    # key-side and query-side column slices: each slice gets its own PSUM
    # bank (matmul output must not cross a bank) and stays >= 256 wide.
    W = Vp if Vp <= 512 else Vp // 2
    QS = [(i * W, W) for i in range(Vp // W)]
    NQ = len(QS)
    Wq = Vq if Vq <= 512 else Vq // 2
    QSq = [(i * Wq, Wq) for i in range(Vq // Wq)]
    NQq = len(QSq)
    KCq = -(-Vq // 128)  # query-row chunks for the output projection
    NS = [(0, 512), (512, 512)]  # output D column halves

    nc = bass.Bass(trn_type="TRN2", target_bir_lowering=False, debug=False,
                   num_devices=NCORES)
    xt = nc.declare_dram_parameter("xt", [D, Vp], BF16, isOutput=False).ap()
    wqk = nc.declare_dram_parameter("wqk", [D, 1024], BF16, isOutput=False).ap()
    wv = nc.declare_dram_parameter("wv", [D, 512], BF16, isOutput=False).ap()
    wo = nc.declare_dram_parameter("wo", [512, D], BF16, isOutput=False).ap()
    biasv = nc.declare_dram_parameter("biasv", [128, KC], F32, isOutput=False).ap()
    onesr = nc.declare_dram_parameter("onesr", [1, 64], F32R, isOutput=False).ap()
    y = nc.declare_dram_parameter("y", [Vq, D], F32, isOutput=True).ap()

    with TileContext(nc) as tc:
        with tc.tile_pool(name="consts", bufs=1) as consts, \
             tc.tile_pool(name="xsb", bufs=1) as xpool, \
             tc.tile_pool(name="wqk", bufs=1) as wqkpool, \
             tc.tile_pool(name="wv", bufs=1) as wvpool, \
             tc.tile_pool(name="wo", bufs=1) as wopool, \
             tc.tile_pool(name="qk", bufs=1) as qkpool, \
             tc.tile_pool(name="vp", bufs=1) as vppool, \
             tc.tile_pool(name="pt", bufs=2 * KC + 6) as ptpool, \
             tc.tile_pool(name="at", bufs=1) as atpool, \
             tc.tile_pool(name="norm", bufs=3) as npool, \
             tc.tile_pool(name="ysb", bufs=2) as ypool:

            bias_sb = consts.tile([128, KC], F32)
            ones_sb = consts.tile([65, 64], F32R)
            nc.sync.dma_start(out=bias_sb[:], in_=biasv[:])
            # ones row lives at partition 64 so the K=1 broadcast matmul's
            # lhsT base partition matches the denominator row of osb.
            nc.sync.dma_start(out=ones_sb[64:65, :], in_=onesr[:])

            # ---- input DMAs: interleave x_k / wqk_k on two engine queues so
            # the qk pair-0 accumulation chain can pipeline with arrival;
            # wv / wo go on the vector engine's queues in parallel. ----
            xsb, wqk_sb, wv_sb, wo_sb = [], [], [], []
            for k in range(8):
                t = xpool.tile([128, Vp], BF16, tag=f"x{k}", name=f"x_{k}")
                nc.scalar.dma_start(out=t[:], in_=xt[k * 128:(k + 1) * 128, :])
                xsb.append(t)
                wt = wqkpool.tile([128, 1024], BF16, tag=f"wqk{k}",
                                  name=f"wqk_{k}")
                nc.sync.dma_start(out=wt[:], in_=wqk[k * 128:(k + 1) * 128, :])
                wqk_sb.append(wt)
            for k in range(8):
                wt = wvpool.tile([128, 512], BF16, tag=f"wv{k}", name=f"wv_{k}")
                nc.gpsimd.dma_start(out=wt[:],
                                    in_=wv[k * 128:(k + 1) * 128, :])
                wv_sb.append(wt)
            for j in range(4):
                wt = wopool.tile([128, 1024], BF16, tag=f"wo{j}")
                nc.gpsimd.dma_start(out=wt[:],
                                    in_=wo[j * 128:(j + 1) * 128, :])
                wo_sb.append(wt)

            at2 = [atpool.tile([128, Vq], BF16, tag=f"at{j}", name=f"at2_{j}")
                   for j in range(4)]
            qk_sb = [None] * 8
            vp_sb = []

            with tc.tile_pool(name="qkps", bufs=1, space="PSUM") as qkps:

                def make_pair_items(p):
                    """Small PE work items for qk projection pair p (m=p is
                    the q chunk, m=4+p the k chunk). PSUM tile allocation is
                    lazy so ring order matches use order."""
                    items = []
                    for m in (p, 4 + p):
                        isq = m < 4
                        mQS, mNQ, mV = (QSq, NQq, Vq) if isq else (QS, NQ, Vp)
                        mW = Wq if isq else W
                        hold = {}
                        for qi, (n0, nw) in enumerate(mQS):
                            for k0 in (0, 4):
                                def mm4(m=m, qi=qi, n0=n0, nw=nw, k0=k0,
                                        hold=hold):
                                    if "ps" not in hold:
                                        hold["ps"] = qkps.tile(
                                            [128, 2, 512], F32, tag="qkp",
                                            name=f"qkp_{m}")
                                    for k in range(k0, k0 + 4):
                                        nc.tensor.matmul(
                                            hold["ps"][:, qi, 0:nw],
                                            lhsT=wqk_sb[k][:,
                                                           m * 128:(m + 1) * 128],
                                            rhs=xsb[k][:, n0:n0 + nw],
                                            start=(k == 0), stop=(k == 7))
                                items.append(mm4)

                        def evac(m=m, hold=hold, mV=mV, mNQ=mNQ, mW=mW):
                            qt = qkpool.tile([128, mV], BF16, tag=f"qk{m}",
                                             name=f"qk_{m}")
                            qtv = qt[:].rearrange("p (q w) -> p q w", q=mNQ)
                            nc.vector.tensor_copy(out=qtv,
                                                  in_=hold["ps"][:, :, 0:mW])
                            qk_sb[m] = qt
                        items.append(evac)
                    return items

                # prologue: qk pair 0 emitted inline (pipelines with the
                # interleaved x_k/wqk_k DMA arrivals), then V' in its own
                # PSUM scope.
                for item in make_pair_items(0):
                    item()

                with tc.tile_pool(name="vps", bufs=2, space="PSUM") as vps:
                    for c in range(KC):
                        ps = vps.tile([128, 512], F32, tag="vpp")
                        for k in range(8):
                            nc.tensor.matmul(
                                ps[:],
                                lhsT=xsb[k][:, c * 128:(c + 1) * 128],
                                rhs=wv_sb[k][:],
                                start=(k == 0), stop=(k == 7))
                        vt = vppool.tile([128, 8, 65], BF16, tag=f"vp{c}")
                        nc.vector.tensor_copy(
                            out=vt[:, :, 0:64],
                            in_=ps[:].rearrange("p (h d) -> p h d", h=8))
                        nc.gpsimd.memset(vt[:, :, 64:65], 1.0)
                        vp_sb.append(vt)

                with tc.tile_pool(name="stps", bufs=2, space="PSUM") as stps, \
                     tc.tile_pool(name="otps", bufs=1, space="PSUM") as otps:

                    # background PE work: qk pairs 1-3, drip-fed into stage 2
                    bg = []
                    for p in (1, 2, 3):
                        bg.extend(make_pair_items(p))
                    bg_i = [0]

                    def inject(n):
                        for _ in range(n):
                            if bg_i[0] < len(bg):
                                bg[bg_i[0]]()
                                bg_i[0] += 1

                    # ---- stage 2: per head-pair attention pipeline ----
                    for hp in range(4):
                        qt = qk_sb[hp]
                        kt = qk_sb[4 + hp]
                        pts = {0: [None] * KC, 1: [None] * KC}
                        ot = {}

                        def emit_ot(h, sub, c, ot=ot, pts=pts):
                            for qi, (n0, nw) in enumerate(QSq):
                                nc.tensor.matmul(
                                    ot[sub][:, qi, 0:nw],
                                    lhsT=vp_sb[c][:, h, :],
                                    rhs=pts[sub][c][:, n0:n0 + nw],
                                    start=(c == 0), stop=(c == KC - 1))

                        def norm_a(sub, ot=ot):
                            # single copy of numerators + denominator row to
                            # SBUF; frees the O^T PSUM tile immediately.
                            osb = npool.tile([65, Vq], F32R, tag="osb")
                            nc.vector.tensor_copy(
                                out=osb[:].rearrange("p (q w) -> p q w",
                                                     q=NQq),
                                in_=ot[sub][0:65, :, 0:Wq])
                            return osb

                        def norm_b(h, sub, osb, hp=hp):
                            # broadcast denom row across 64 partitions with a
                            # K=1 ones-matmul (f32r, full rate), reciprocal on
                            # the wide tile (128-lane), multiply -> A^T bf16.
                            rb = stps.tile([128, NQq, 512], F32, tag="st",
                                           name=f"rb_{h}")
                            for qi, (n0, nw) in enumerate(QSq):
                                nc.tensor.matmul(rb[0:64, qi, 0:nw],
                                                 lhsT=ones_sb[64:65, :],
                                                 rhs=osb[64:65, n0:n0 + nw],
                                                 start=True, stop=True)
                            rbc = npool.tile([64, Vq], F32R, tag="rbc")
                            with nc.allow_low_precision(
                                    reason="f32r output is bit-identical f32"):
                                nc.vector.reciprocal(
                                    out=rbc[:].rearrange("p (q w) -> p q w",
                                                         q=NQq),
                                    in_=rb[0:64, :, 0:Wq])
                            ov = osb[0:64, :].rearrange("p (q w) -> p q w",
                                                        q=NQq)
                            rv = rbc[:].rearrange("p (q w) -> p q w", q=NQq)
                            if sub == 0:
                                nc.vector.tensor_mul(
                                    at2[hp][0:64, :].rearrange(
                                        "p (q w) -> p q w", q=NQq), ov, rv)
                            else:
                                tmp = npool.tile([64, Vq], BF16, tag="odd")
                                nc.vector.tensor_mul(
                                    tmp[:].rearrange("p (q w) -> p q w",
                                                     q=NQq), ov, rv)
                                nc.sync.dma_start(out=at2[hp][64:128, :],
                                                  in_=tmp[:])

                        # S^T + EXP pipeline over key chunks; even head's O^T
                        # trails by 2 chunks; odd head's O^T runs after.
                        for c in range(KC):
                            st = {}
                            for sub in (0, 1):
                                st[sub] = stps.tile([128, NQq, 512], F32,
                                                    tag="st",
                                                    name=f"st_{hp}_{sub}_{c}")
                            # adjacent emission on disjoint PE row groups
                            for qi, (n0, nw) in enumerate(QSq):
                                for sub in (0, 1):
                                    lo = sub * 64
                                    nc.tensor.matmul(
                                        st[sub][:, qi, 0:nw],
                                        lhsT=kt[lo:lo + 64,
                                                c * 128:(c + 1) * 128],
                                        rhs=qt[lo:lo + 64, n0:n0 + nw],
                                        start=True, stop=True)
                            for sub in (0, 1):
                                pt = ptpool.tile([128, Vq], BF16, tag="pt",
                                                 name=f"pt_{hp}_{sub}_{c}")
                                nc.scalar.activation(
                                    out=pt[:].rearrange("p (q w) -> p q w",
                                                        q=NQq),
                                    in_=st[sub][:, :, 0:Wq], func=AF.Exp,
                                    bias=bias_sb[:, c:c + 1], scale=1.0)
                                pts[sub][c] = pt
                            if c == 0:
                                ot[0] = otps.tile([65, NQq, 512], F32,
                                                  tag="ot",
                                                  name=f"ot_{2 * hp}")
                            if c >= 2:
                                emit_ot(2 * hp, 0, c - 2)
                            inject(2)

                        for cc in range(max(0, KC - 2), KC):
                            emit_ot(2 * hp, 0, cc)
                        osb0 = norm_a(0)
                        inject(1)
                        ot[1] = otps.tile([65, NQq, 512], F32, tag="ot",
                                          name=f"ot_{2 * hp + 1}")
                        for c in range(min(2, KC)):
                            emit_ot(2 * hp + 1, 1, c)
                        norm_b(2 * hp, 0, osb0)
                        for c in range(2, KC):
                            emit_ot(2 * hp + 1, 1, c)
                            if c % 2 == 1:
                                inject(1)
                        osb1 = norm_a(1)
                        inject(1)
                        norm_b(2 * hp + 1, 1, osb1)
                        inject(1)
                    inject(len(bg))

            # ---- stage 3: y[qc] = sum_j Apair_j @ wopair_j  (K=128) ----
            with tc.tile_pool(name="yps", bufs=2, space="PSUM") as yps:
                for qc in range(KCq):
                    mw = min(128, Vq - qc * 128)
                    yp = yps.tile([128, 1024], F32, tag="yp")
                    for j in range(4):
                        for (n0, nw) in NS:
                            nc.tensor.matmul(
                                yp[0:mw, n0:n0 + nw],
                                lhsT=at2[j][:, qc * 128:qc * 128 + mw],
                                rhs=wo_sb[j][:, n0:n0 + nw],
                                start=(j == 0), stop=(j == 3))
                    ysb = ypool.tile([128, 1024], F32, tag="ysb")
                    nc.scalar.copy(out=ysb[0:mw, :], in_=yp[0:mw, :])
                    nc.scalar.dma_start(out=y[qc * 128:qc * 128 + mw, :],
                                        in_=ysb[0:mw, :])
    return nc


def kernel(x, mask, w_qkv, w_out, b_out):
    _install_patches()
    from concourse.bass_utils import run_bass_kernel_spmd

    x = np.asarray(x, dtype=np.float32)
    mask = np.asarray(mask, dtype=np.float32)
    w_qkv = np.asarray(w_qkv, dtype=np.float32)
    w_out = np.asarray(w_out, dtype=np.float32)
    b_out = np.asarray(b_out, dtype=np.float32)

    idx = [np.nonzero(mask[b] != 0.0)[0] for b in range(B)]
    nv = [len(i) for i in idx]
    Vp = max(128, int(-(-max(nv) // 128)) * 128)
    Vq = max(128, int(-(-max(nv) // 32)) * 32)
    if max(nv) == 0:
        return np.broadcast_to(b_out, (B, N, D)).astype(np.float32).copy()

    import ml_dtypes
    bf16 = ml_dtypes.bfloat16

    scale = float(DH) ** -0.5
    G = 512  # features per head-group
    wqk_g, wv_g, wo_g = [], [], []
    for g in range(2):
        wq = w_qkv[:, g * G:(g + 1) * G] * scale
        wk = w_qkv[:, 1024 + g * G:1024 + (g + 1) * G]
        wqk_g.append(np.ascontiguousarray(
            np.concatenate([wq, wk], axis=1).astype(bf16)))
        wv_g.append(np.ascontiguousarray(
            w_qkv[:, 2048 + g * G:2048 + (g + 1) * G].astype(bf16)))
        wo_g.append(np.ascontiguousarray(
            w_out[g * G:(g + 1) * G, :].astype(bf16)))

    xt_b, bias_b = [], []
    for b in range(B):
        pad = Vp - nv[b]
        idxp = np.concatenate([idx[b], np.zeros(pad, dtype=np.int64)])
        xg = x[b][idxp, :]
        xt_b.append(np.ascontiguousarray(xg.T.astype(bf16)))
        bv = np.concatenate([
            np.zeros(nv[b], dtype=np.float32),
            np.full(pad, PAD_BIAS, dtype=np.float32)])
        bias_b.append(np.ascontiguousarray(bv.reshape(-1, 128).T))
    onesr = np.ones((1, 64), dtype=np.float32)

    nc = _build_program(Vp, Vq)
    in_maps = []
    for core in range(NCORES):
        b, g = core // 2, core % 2
        in_maps.append({
            "xt": xt_b[b], "wqk": wqk_g[g], "wv": wv_g[g], "wo": wo_g[g],
            "biasv": bias_b[b], "onesr": onesr,
        })

    trace = bool(os.environ.get("BASSK_TRACE"))
    if trace:
        _install_profile_hook()
    res = run_bass_kernel_spmd(nc, in_maps, list(range(NCORES)), trace=trace)
    global last_exec_time_ns
    last_exec_time_ns = res.exec_time_ns

    out = np.zeros((B, N, D), dtype=np.float32)
    for b in range(B):
        yb = res.results[2 * b]["y"] + res.results[2 * b + 1]["y"]
        out[b][idx[b]] = yb[:nv[b]]
    out += b_out
    return out


last_exec_time_ns = None


def _install_profile_hook():
    import types
    import antenv
    if 'antenv.axon_hooks' in sys.modules:
        return
    import trn_agent_boot.trn_boot as tb
    _hook = tb._ntff_profile_via_ctypes('/opt/axon/libaxon_pjrt.so')
    mod = types.ModuleType('antenv.axon_hooks')
    mod.get_axon_ntff_profile_hook = lambda: _hook
    mod.set_axon_ntff_profile_hook = lambda h: None
    sys.modules['antenv.axon_hooks'] = mod
    antenv.axon_hooks = mod
    bass_utils.upload_artifacts = lambda tmpdir: "local://skipped"


# revision 12
# speedup vs baseline: 1.4767x; 1.4158x over previous
"""Masked multi-head attention block (B=4, N=1024, D=1024, H=16, DH=64) on 8
Trainium2 NeuronCores.

Sharding: core (b, g) = 2*b + g handles batch b and head-group g (8 of 16
heads). Each core computes qkv projections for its heads, attention, and its
partial output projection; the host sums the two head-group partials per batch.

Mask handling: the host gathers only the valid tokens per batch (padded to a
multiple of 128 with key-bias -30000 on the pad), so the device computes a
dense unmasked attention over ~half the sequence; invalid token rows of the
output are b_out.

v4 layout (per core, Vp = padded key count, Vq = padded query count):
  All inputs bf16.  x / wv / wo are packed on the host so each SBUF partition
  row is one contiguous DMA descriptor (10KB/8KB/8KB) instead of 8 small ones
  -- the input wall is descriptor-bound, not byte-bound.  wqk stays per-k so
  the pair-0 accumulation chain pipelines with chunk arrival.
  V'   [Vp, 8, 65]   values per head + ones column (softmax denominator)
  S^T  [Vp,Vq]/head  = K_chunk @ Q^T; exp on ACT with per-partition pad bias
  O^T  [65, Vq]/head = V'^T @ P^T accumulated over key chunks (row 64=denom)
  norm: one [65,Vq] PSUM->SBUF copy (numerators+denom, frees PSUM), K=1
        ones-matmul broadcasts the denom row across 64 partitions into PSUM,
        reciprocal_approx_fast on the wide tile (128-lane DVE), multiply.
  y    [Vq, D] = sum_h A_h @ w_out_h  (K=128 accumulating matmuls)

Scheduling: qk pair 0 and V' interleave k-outer through the DMA arrivals;
pairs 1-3 are background PE work drip-fed between attention chunks so the PE
never idles long enough for the HAM clock gate to re-throttle.
PSUM: prologue qkps 2 + vps 5 banks; stage 2 qkps 2 + stps 4 + otps 2.
"""
import json
import os
import sys

import numpy as np

sys.path.insert(0, "/opt/trn_rl_repo")

import concourse.bass as bass
import concourse.mybir as mybir
from concourse.tile import TileContext
from concourse import bass_utils

F32 = mybir.dt.float32
F32R = mybir.dt.float32r
BF16 = mybir.dt.bfloat16
AF = mybir.ActivationFunctionType

B, N, D, H, DH = 4, 1024, 1024, 16, 64
NCORES = 8
PAD_BIAS = -30000.0


def _install_patches():
    """The walrus build in this container accepts only one semaphore wait per
    instruction; hoist extra waits onto same-engine NoOps in the BIR json."""
    if getattr(bass.Bass, "_split_waits_patched", False):
        return
    orig = bass.Bass.to_json_bytes

    def to_json_bytes_split(self, *a, **k):
        j = json.loads(orig(self, *a, **k))
        for fn in j.get("functions", []):
            for bb in fn.get("blocks", []):
                out = []
                for ins in bb.get("instructions", []):
                    si = ins.get("sync_info") or {}
                    waits = si.get("on_wait") or []
                    if len(waits) > 1:
                        for i, w in enumerate(waits[:-1]):
                            out.append({
                                "debug": ins.get("debug", 0),
                                "engine": ins["engine"],
                                "ins": [],
                                "name": f"{ins['name']}_sw{i}",
                                "opcode": "NoOp",
                                "outs": [],
                                "text_hint": "splitw",
                                "sync_info": {"on_update": [], "on_wait": [w]},
                            })
                        si["on_wait"] = [waits[-1]]
                    out.append(ins)
                bb["instructions"] = out
        return json.dumps(j).encode()

    bass.Bass.to_json_bytes = to_json_bytes_split

    def _drain_and_barrier(self, tick_clock, wait_clock):
        import re as _re
        import bass_rust as _br
        from concourse.vector_clock import ScopedClock as _SC
        gc = tick_clock.global_clock
        comps = eval(_re.match(r"VectorClock\((\[.*\])\)", repr(gc)).group(1))
        for i, v in enumerate(comps):
            if v <= 0:
                continue
            sub = [0] * len(comps)
            sub[i] = v
            nop = self.nc.sync.nop(nofuse=True, hint="final_wait")
            wait_clock.add_sem_waits(nop.ins, _SC({None: _br.VectorClock(sub)}))
        self.nc.sync.drain()
        self.nc.all_engine_barrier()
        assert self.sems is not None
        popped = self.nc._tile_sem_poison_stack.pop()
        assert popped is self._sem_poison
        self.nc.clear_and_free_semaphores(list(self.sems.allocated().values()))

    TileContext._drain_and_barrier = _drain_and_barrier
    bass.Bass._split_waits_patched = True


def _build_program(Vp, Vq):
    KC = Vp // 128
    # key-side and query-side column slices: each slice gets its own PSUM
    # bank (matmul output must not cross a bank) and stays >= 256 wide.
    W = Vp if Vp <= 512 else Vp // 2
    QS = [(i * W, W) for i in range(Vp // W)]
    NQ = len(QS)
    Wq = Vq if Vq <= 512 else Vq // 2
    QSq = [(i * Wq, Wq) for i in range(Vq // Wq)]
    NQq = len(QSq)
    KCq = -(-Vq // 128)  # query-row chunks for the output projection
    NS = [(0, 512), (512, 512)]  # output D column halves

    nc = bass.Bass(trn_type="TRN2", target_bir_lowering=False, debug=False,
                   num_devices=NCORES)
    xt = nc.declare_dram_parameter("xt", [128, 8 * Vp], BF16,
                                   isOutput=False).ap()
    wqk = nc.declare_dram_parameter("wqk", [D, 1024], BF16, isOutput=False).ap()
    wv = nc.declare_dram_parameter("wv", [128, 8 * 512], BF16,
                                   isOutput=False).ap()
    wo = nc.declare_dram_parameter("wo", [128, 4 * 1024], BF16,
                                   isOutput=False).ap()
    biasv = nc.declare_dram_parameter("biasv", [128, KC], F32, isOutput=False).ap()
    onesr = nc.declare_dram_parameter("onesr", [1, 64], F32R, isOutput=False).ap()
    y = nc.declare_dram_parameter("y", [Vq, D], F32, isOutput=True).ap()

    with TileContext(nc) as tc:
        with tc.tile_pool(name="consts", bufs=1) as consts, \
             tc.tile_pool(name="xsb", bufs=1) as xpool, \
             tc.tile_pool(name="wqk", bufs=1) as wqkpool, \
             tc.tile_pool(name="wv", bufs=1) as wvpool, \
             tc.tile_pool(name="wo", bufs=1) as wopool, \
             tc.tile_pool(name="qk", bufs=1) as qkpool, \
             tc.tile_pool(name="vp", bufs=1) as vppool, \
             tc.tile_pool(name="pt", bufs=2 * KC + 6) as ptpool, \
             tc.tile_pool(name="at", bufs=1) as atpool, \
             tc.tile_pool(name="norm", bufs=3) as npool, \
             tc.tile_pool(name="ysb", bufs=2) as ypool:

            bias_sb = consts.tile([128, KC], F32)
            ones_sb = consts.tile([65, 64], F32R)
            nc.sync.dma_start(out=bias_sb[:], in_=biasv[:])
            # ones row lives at partition 64 so the K=1 broadcast matmul's
            # lhsT base partition matches the denominator row of osb.
            nc.sync.dma_start(out=ones_sb[64:65, :], in_=onesr[:])

            # ---- input DMAs.  scalar ring: x half 0, wv, x half 1, wo
            # (big merged descriptors); sync ring: wqk per-k (pipelines the
            # pair-0 accumulation chain). ----
            xsb_t = xpool.tile([128, 8, Vp], BF16, tag="x", name="x_all")
            wv_sb_t = wvpool.tile([128, 8, 512], BF16, tag="wv", name="wv_all")
            wo_sb_t = wopool.tile([128, 4, 1024], BF16, tag="wo", name="wo_all")
            nc.scalar.dma_start(out=xsb_t[:, 0:4, :], in_=xt[:, 0:4 * Vp])
            nc.scalar.dma_start(out=wv_sb_t[:], in_=wv[:])
            nc.scalar.dma_start(out=xsb_t[:, 4:8, :], in_=xt[:, 4 * Vp:8 * Vp])
            nc.scalar.dma_start(out=wo_sb_t[:], in_=wo[:])
            wqk_sb = []
            for k in range(8):
                wt = wqkpool.tile([128, 1024], BF16, tag=f"wqk{k}",
                                  name=f"wqk_{k}")
                nc.sync.dma_start(out=wt[:], in_=wqk[k * 128:(k + 1) * 128, :])
                wqk_sb.append(wt)

            at2 = [atpool.tile([128, Vq], BF16, tag=f"at{j}", name=f"at2_{j}")
                   for j in range(4)]
            qk_sb = [None] * 8
            vp_sb = []

            def make_pair_items(p, qkps):
                """Small PE work items for qk projection pair p (m=p is the q
                chunk, m=4+p the k chunk). PSUM tile allocation is lazy so
                ring order matches use order."""
                items = []
                for m in (p, 4 + p):
                    isq = m < 4
                    mQS, mNQ, mV = (QSq, NQq, Vq) if isq else (QS, NQ, Vp)
                    mW = Wq if isq else W
                    hold = {}
                    for qi, (n0, nw) in enumerate(mQS):
                        for k0 in (0, 4):
                            def mm4(m=m, qi=qi, n0=n0, nw=nw, k0=k0,
                                    hold=hold):
                                if "ps" not in hold:
                                    hold["ps"] = qkps.tile(
                                        [128, 2, 512], F32, tag="qkp",
                                        name=f"qkp_{m}")
                                for k in range(k0, k0 + 4):
                                    nc.tensor.matmul(
                                        hold["ps"][:, qi, 0:nw],
                                        lhsT=wqk_sb[k][:,
                                                       m * 128:(m + 1) * 128],
                                        rhs=xsb_t[:, k, n0:n0 + nw],
                                        start=(k == 0), stop=(k == 7))
                            items.append(mm4)

                    def evac(m=m, hold=hold, mV=mV, mNQ=mNQ, mW=mW):
                        qt = qkpool.tile([128, mV], BF16, tag=f"qk{m}",
                                         name=f"qk_{m}")
                        qtv = qt[:].rearrange("p (q w) -> p q w", q=mNQ)
                        nc.vector.tensor_copy(out=qtv,
                                              in_=hold["ps"][:, :, 0:mW])
                        qk_sb[m] = qt
                    items.append(evac)
                return items

            with tc.tile_pool(name="qkps", bufs=1, space="PSUM") as qkps:

                # ---- prologue: pair 0 and V' interleaved k-outer so compute
                # tracks the DMA arrivals chunk by chunk. ----
                with tc.tile_pool(name="vps", bufs=KC, space="PSUM") as vps:
                    p0ps = {}
                    vpps = [vps.tile([128, 512], F32, tag="vpp",
                                     name=f"vpp_{c}") for c in range(KC)]
                    for m in (0, 4):
                        p0ps[m] = qkps.tile([128, 2, 512], F32, tag="qkp",
                                            name=f"qkp_{m}")
                    for k in range(8):
                        for m in (0, 4):
                            mQS = QSq if m < 4 else QS
                            for qi, (n0, nw) in enumerate(mQS):
                                nc.tensor.matmul(
                                    p0ps[m][:, qi, 0:nw],
                                    lhsT=wqk_sb[k][:, m * 128:(m + 1) * 128],
                                    rhs=xsb_t[:, k, n0:n0 + nw],
                                    start=(k == 0), stop=(k == 7))
                        for c in range(KC):
                            nc.tensor.matmul(
                                vpps[c][:],
                                lhsT=xsb_t[:, k, c * 128:(c + 1) * 128],
                                rhs=wv_sb_t[:, k, :],
                                start=(k == 0), stop=(k == 7))
                    for m in (0, 4):
                        isq = m < 4
                        mV, mNQ, mW = (Vq, NQq, Wq) if isq else (Vp, NQ, W)
                        qt = qkpool.tile([128, mV], BF16, tag=f"qk{m}",
                                         name=f"qk_{m}")
                        nc.vector.tensor_copy(
                            out=qt[:].rearrange("p (q w) -> p q w", q=mNQ),
                            in_=p0ps[m][:, :, 0:mW])
                        qk_sb[m] = qt
                    for c in range(KC):
                        vt = vppool.tile([128, 8, 65], BF16, tag=f"vp{c}")
                        nc.vector.tensor_copy(
                            out=vt[:, :, 0:64],
                            in_=vpps[c][:].rearrange("p (h d) -> p h d", h=8))
                        nc.gpsimd.memset(vt[:, :, 64:65], 1.0)
                        vp_sb.append(vt)

                with tc.tile_pool(name="stps", bufs=2, space="PSUM") as stps, \
                     tc.tile_pool(name="otps", bufs=1, space="PSUM") as otps:

                    # background PE work: qk pairs 1-3, drip-fed into stage 2
                    bg = []
                    for p in (1, 2, 3):
                        bg.extend(make_pair_items(p, qkps))
                    bg_i = [0]

                    def inject(n):
                        for _ in range(n):
                            if bg_i[0] < len(bg):
                                bg[bg_i[0]]()
                                bg_i[0] += 1

                    # ---- stage 2: per head-pair attention pipeline ----
                    for hp in range(4):
                        qt = qk_sb[hp]
                        kt = qk_sb[4 + hp]
                        pts = {0: [None] * KC, 1: [None] * KC}
                        ot = {}

                        def emit_ot(h, sub, c, ot=ot, pts=pts):
                            for qi, (n0, nw) in enumerate(QSq):
                                nc.tensor.matmul(
                                    ot[sub][:, qi, 0:nw],
                                    lhsT=vp_sb[c][:, h, :],
                                    rhs=pts[sub][c][:, n0:n0 + nw],
                                    start=(c == 0), stop=(c == KC - 1))

                        def osb_evac(sub, ot=ot):
                            # numerators -> SBUF; frees the O^T PSUM tile
                            # (together with the denominator ln read).
                            osb = npool.tile([64, Vq], F32, tag="osb")
                            nc.vector.tensor_copy(
                                out=osb[:].rearrange("p (q w) -> p q w",
                                                     q=NQq),
                                in_=ot[sub][0:64, :, 0:Wq])
                            return osb

                        def denom_recip(h, sub, ot=ot):
                            # 1/denom via exp(-ln d) on ACT (same table set
                            # as the softmax exp); stays on partition 64 so
                            # the broadcast matmul's operand bases match.
                            rln = npool.tile([65, Vq], F32, tag="rln")
                            nc.scalar.activation(
                                out=rln[64:65, :].rearrange(
                                    "p (q w) -> p q w", q=NQq),
                                in_=ot[sub][64:65, :, 0:Wq], func=AF.Ln)
                            rex = npool.tile([65, Vq], F32R, tag="rex")
                            nc.scalar.activation(out=rex[64:65, :],
                                                 in_=rln[64:65, :],
                                                 func=AF.Exp, scale=-1.0)
                            return rex

                        def bcast(h, rex):
                            # K=1 ones-matmul broadcast of the 1/denom row
                            rb = otps.tile([65, NQq, 512], F32, tag="ot",
                                           name=f"rb_{h}")
                            for qi, (n0, nw) in enumerate(QSq):
                                nc.tensor.matmul(
                                    rb[0:64, qi, 0:nw],
                                    lhsT=ones_sb[64:65, :],
                                    rhs=rex[64:65, n0:n0 + nw],
                                    start=True, stop=True)
                            return rb

                        def mul_at(h, sub, osb, rb, hp=hp):
                            ov = osb[:].rearrange("p (q w) -> p q w", q=NQq)
                            rv = rb[0:64, :, 0:Wq]
                            if sub == 0:
                                nc.vector.tensor_mul(
                                    at2[hp][0:64, :].rearrange(
                                        "p (q w) -> p q w", q=NQq), ov, rv)
                            else:
                                tmp = npool.tile([64, Vq], BF16, tag="odd")
                                nc.vector.tensor_mul(
                                    tmp[:].rearrange("p (q w) -> p q w",
                                                     q=NQq), ov, rv)
                                nc.sync.dma_start(out=at2[hp][64:128, :],
                                                  in_=tmp[:])

                        # S^T + EXP pipeline over key chunks; even head's O^T
                        # trails by 2 chunks; odd head's O^T runs after.
                        for c in range(KC):
                            st = {}
                            for sub in (0, 1):
                                st[sub] = stps.tile([128, NQq, 512], F32,
                                                    tag="st",
                                                    name=f"st_{hp}_{sub}_{c}")
                            # adjacent emission on disjoint PE row groups
                            for qi, (n0, nw) in enumerate(QSq):
                                for sub in (0, 1):
                                    lo = sub * 64
                                    nc.tensor.matmul(
                                        st[sub][:, qi, 0:nw],
                                        lhsT=kt[lo:lo + 64,
                                                c * 128:(c + 1) * 128],
                                        rhs=qt[lo:lo + 64, n0:n0 + nw],
                                        start=True, stop=True)
                            for sub in (0, 1):
                                pt = ptpool.tile([128, Vq], BF16, tag="pt",
                                                 name=f"pt_{hp}_{sub}_{c}")
                                nc.scalar.activation(
                                    out=pt[:].rearrange("p (q w) -> p q w",
                                                        q=NQq),
                                    in_=st[sub][:, :, 0:Wq], func=AF.Exp,
                                    bias=bias_sb[:, c:c + 1], scale=1.0)
                                pts[sub][c] = pt
                            if c == 0:
                                ot[0] = otps.tile([65, NQq, 512], F32,
                                                  tag="ot",
                                                  name=f"ot_{2 * hp}")
                            if c >= 2:
                                emit_ot(2 * hp, 0, c - 2)
                            inject(2)

                        for cc in range(max(0, KC - 2), KC):
                            emit_ot(2 * hp, 0, cc)
                        rex0 = denom_recip(2 * hp, 0)
                        osb0 = osb_evac(0)
                        rb0 = bcast(2 * hp, rex0)
                        inject(1)
                        ot[1] = otps.tile([65, NQq, 512], F32, tag="ot",
                                          name=f"ot_{2 * hp + 1}")
                        for c in range(min(2, KC)):
                            emit_ot(2 * hp + 1, 1, c)
                        mul_at(2 * hp, 0, osb0, rb0)
                        for c in range(2, KC):
                            emit_ot(2 * hp + 1, 1, c)
                            if c % 2 == 1:
                                inject(1)
                        rex1 = denom_recip(2 * hp + 1, 1)
                        osb1 = osb_evac(1)
                        rb1 = bcast(2 * hp + 1, rex1)
                        inject(1)
                        mul_at(2 * hp + 1, 1, osb1, rb1)
                        inject(1)
                    inject(len(bg))

            # ---- stage 3: y[qc] = sum_j Apair_j @ wopair_j  (K=128) ----
            with tc.tile_pool(name="yps", bufs=2, space="PSUM") as yps:
                for qc in range(KCq):
                    mw = min(128, Vq - qc * 128)
                    yp = yps.tile([128, 1024], F32, tag="yp")
                    for j in range(4):
                        for (n0, nw) in NS:
                            nc.tensor.matmul(
                                yp[0:mw, n0:n0 + nw],
                                lhsT=at2[j][:, qc * 128:qc * 128 + mw],
                                rhs=wo_sb_t[:, j, n0:n0 + nw],
                                start=(j == 0), stop=(j == 3))
                    ysb = ypool.tile([128, 1024], F32, tag="ysb")
                    nc.vector.tensor_copy(out=ysb[0:mw, :], in_=yp[0:mw, :])
                    nc.scalar.dma_start(out=y[qc * 128:qc * 128 + mw, :],
                                        in_=ysb[0:mw, :])
    return nc


def kernel(x, mask, w_qkv, w_out, b_out):
    _install_patches()
    from concourse.bass_utils import run_bass_kernel_spmd

    x = np.asarray(x, dtype=np.float32)
    mask = np.asarray(mask, dtype=np.float32)
    w_qkv = np.asarray(w_qkv, dtype=np.float32)
    w_out = np.asarray(w_out, dtype=np.float32)
    b_out = np.asarray(b_out, dtype=np.float32)

    idx = [np.nonzero(mask[b] != 0.0)[0] for b in range(B)]
    nv = [len(i) for i in idx]
    Vp = max(128, int(-(-max(nv) // 128)) * 128)
    Vq = max(128, int(-(-max(nv) // 32)) * 32)
    if max(nv) == 0:
        return np.broadcast_to(b_out, (B, N, D)).astype(np.float32).copy()

    import ml_dtypes
    bf16 = ml_dtypes.bfloat16

    def pack_k(a, nk):
        # [nk*128, W] -> [128, nk*W]: partition p holds its nk chunks
        # contiguously, so the DMA moves one big descriptor per partition.
        W_ = a.shape[1]
        return np.ascontiguousarray(
            a.reshape(nk, 128, W_).transpose(1, 0, 2).reshape(128, nk * W_))

    scale = float(DH) ** -0.5
    G = 512  # features per head-group
    wqk_g, wv_g, wo_g = [], [], []
    for g in range(2):
        wq = w_qkv[:, g * G:(g + 1) * G] * scale
        wk = w_qkv[:, 1024 + g * G:1024 + (g + 1) * G]
        wqk_g.append(np.ascontiguousarray(
            np.concatenate([wq, wk], axis=1).astype(bf16)))
        wv_g.append(pack_k(
            w_qkv[:, 2048 + g * G:2048 + (g + 1) * G].astype(bf16), 8))
        wo_g.append(pack_k(w_out[g * G:(g + 1) * G, :].astype(bf16), 4))

    xt_b, bias_b = [], []
    for b in range(B):
        pad = Vp - nv[b]
        idxp = np.concatenate([idx[b], np.zeros(pad, dtype=np.int64)])
        xg = x[b][idxp, :]
        xt_b.append(pack_k(np.ascontiguousarray(xg.T.astype(bf16)), 8))
        bv = np.concatenate([
            np.zeros(nv[b], dtype=np.float32),
            np.full(pad, PAD_BIAS, dtype=np.float32)])
        bias_b.append(np.ascontiguousarray(bv.reshape(-1, 128).T))
    onesr = np.ones((1, 64), dtype=np.float32)

    nc = _build_program(Vp, Vq)
    in_maps = []
    for core in range(NCORES):
        b, g = core // 2, core % 2
        in_maps.append({
            "xt": xt_b[b], "wqk": wqk_g[g], "wv": wv_g[g], "wo": wo_g[g],
            "biasv": bias_b[b], "onesr": onesr,
        })

    trace = bool(os.environ.get("BASSK_TRACE"))
    if trace:
        _install_profile_hook()
    res = run_bass_kernel_spmd(nc, in_maps, list(range(NCORES)), trace=trace)
    global last_exec_time_ns
    last_exec_time_ns = res.exec_time_ns

    out = np.zeros((B, N, D), dtype=np.float32)
    for b in range(B):
        yb = res.results[2 * b]["y"] + res.results[2 * b + 1]["y"]
        out[b][idx[b]] = yb[:nv[b]]
    out += b_out
    return out


last_exec_time_ns = None


def _install_profile_hook():
    import types
    import antenv
    if 'antenv.axon_hooks' in sys.modules:
        return
    import trn_agent_boot.trn_boot as tb
    _hook = tb._ntff_profile_via_ctypes('/opt/axon/libaxon_pjrt.so')
    mod = types.ModuleType('antenv.axon_hooks')
    mod.get_axon_ntff_profile_hook = lambda: _hook
    mod.set_axon_ntff_profile_hook = lambda h: None
    sys.modules['antenv.axon_hooks'] = mod
    antenv.axon_hooks = mod
    bass_utils.upload_artifacts = lambda tmpdir: "local://skipped"
